# revision 29
# baseline (speedup 1.0000x reference)
"""Causal cosine-sim attention (qk rmsnorm, scale=8) on 8 trn2 NeuronCores.

Shapes: q,k,v [2,16,2048,64] fp32; out [2,16,2048,64] fp32.
Sharding: 32 (batch, head) pairs -> 4 per core (head-parallel, per the
sharding hint); each core runs an identical SPMD program on its own 4
heads.  Measured ~192us HW exec (from ~253us naive-flash baseline).

Per-core flash-attention-style algorithm (per head):
  preprocess: load Q/K [s,d]; l2-normalize rows (DVE square+reduce,
    rsqrt via Quake-magic + 2 Newton iterations on DVE, keeping ScalarE
    exp-only so its spline-table set loads exactly once); cast to bf16;
    round-trip through a DRAM scratch [s,128] whose two 64-col halves
    are both written, then DMA-xbar-transpose back as qT/kT [128, s]
    bf16 with duplicated partition halves.  The duplication makes the
    S^T matmuls contract over K=128 (computing 2x the dot product,
    halved inside the exp scale): K=64 matmuls do not count as PE-busy
    for the HAM activity monitor and pin the PE clock at 1.2 GHz, while
    full-K matmuls let it reach 2.4 GHz.  V loads as [s,d], cast bf16
    with a ones-column appended (rowsum rides along in the PV matmul).
  attention (j-major over key blocks, i-halves of 1024): per key block
    jb, one S^T tile = kT_jb.T @ qT over the causal i-tail inside the
    half (PSUM, <=1024 cols, 512-col matmuls); one ACT
    exp(4*x + mask_bias) per jb PSUM->SBUF bf16 (mask_bias is the
    key-padding mask as a per-partition additive bias); the diagonal
    128x128 block is multiplied by a lower-triangular 0/1 mask into a
    separate small tile on GpSimd so the main PV chain never waits on
    it; O^T[65, i-half] += V_jb.T @ P^T accumulates in PSUM (col 64 =
    softmax denominator).  Within a bank, only the chronologically
    first matmul may carry start=True (its PSUM has_written clear is
    wider than the written range), so jb==0 stays in emission order.
  epilogue per half: copy O^T to SBUF (DVE), PE-transpose per 128-col
    tile, divide by the rowsum (DVE reciprocal + tensor_scalar), DMA
    out.  preprocess(h+1) is emitted between the two halves of head h
    so its DMA/DVE work hides under attention without delaying the
    half-0 epilogue in the DVE FIFO.
"""

import sys

import numpy as np

try:
    import concourse.bass as bass
except ImportError:
    sys.path.insert(0, "/opt/trn_rl_repo")
    import concourse.bass as bass

import concourse.mybir as mybir
import concourse.tile as tile
from concourse import bacc
from concourse.bass_utils import run_bass_kernel_spmd
from concourse.masks import make_identity

FP32 = mybir.dt.float32
BF16 = mybir.dt.bfloat16

N_CORES = 8
B, H, S, D = 2, 16, 2048, 64
HPC = (B * H) // N_CORES  # heads per core = 4
P = 128
NT = S // P  # 16 key/query blocks
HALF = S // 2
COSINE_SIM_SCALE = 8.0
MASK_NEG = -1e30


def build_nc():
    nc = bacc.Bacc("TRN2", target_bir_lowering=False, debug=False)

    q_d = nc.dram_tensor("q", [HPC, S, D], FP32, kind="ExternalInput")
    k_d = nc.dram_tensor("k", [HPC, S, D], FP32, kind="ExternalInput")
    v_d = nc.dram_tensor("v", [HPC, S, D], FP32, kind="ExternalInput")
    qs_d = nc.dram_tensor("q_scale", [D], FP32, kind="ExternalInput")
    ks_d = nc.dram_tensor("k_scale", [D], FP32, kind="ExternalInput")
    mb_d = nc.dram_tensor("mbias", [HPC, S], FP32, kind="ExternalInput")
    out_d = nc.dram_tensor("out", [HPC, S, D], FP32, kind="ExternalOutput")

    AF = mybir.ActivationFunctionType
    ALU = mybir.AluOpType

    with tile.TileContext(nc) as tc:
        with (
            tc.tile_pool(name="constp", bufs=1) as constp,
            tc.tile_pool(name="dramp", bufs=4, space="DRAM") as dramp,
            tc.tile_pool(name="stagep", bufs=7) as stagep,
            tc.tile_pool(name="sqp", bufs=2) as sqp,
            tc.tile_pool(name="ssp", bufs=12) as ssp,
            tc.tile_pool(name="qnp", bufs=4) as qnp,
            tc.tile_pool(name="qtp", bufs=3) as qtp,
            tc.tile_pool(name="ktp", bufs=3) as ktp,
            tc.tile_pool(name="vbp", bufs=3) as vbp,
            tc.tile_pool(name="mbp", bufs=3) as mbp,
            tc.tile_pool(name="ptp", bufs=5) as ptp,
            tc.tile_pool(name="dtp", bufs=6) as dtp,
            tc.tile_pool(name="otsbp", bufs=2) as otsbp,
            tc.tile_pool(name="osbp", bufs=2) as osbp,
            tc.tile_pool(name="recp", bufs=8) as recp,
            tc.tile_pool(name="stp", bufs=3, space="PSUM") as stp,
            tc.tile_pool(name="otp", bufs=1, space="PSUM") as otp,
        ):
            # ---- constants ----
            tri = constp.tile([P, P], BF16, name="tri")
            nc.gpsimd.memset(tri[:], 1.0)
            # keep where col >= row (P^T layout: row=key j, col=query i)
            nc.gpsimd.affine_select(
                out=tri[:],
                in_=tri[:],
                pattern=[[1, P]],
                channel_multiplier=-1,
                base=0,
                compare_op=ALU.is_ge,
                fill=0.0,
            )
            ident = constp.tile([P, P], FP32, name="ident")
            make_identity(nc, ident[:])
            # q/k per-dim scales duplicated over both partition halves so
            # one tensor_scalar covers the row-packed qT/kT copies
            qscale_sb = constp.tile([P, 1], FP32, name="qscale_sb")
            kscale_sb = constp.tile([P, 1], FP32, name="kscale_sb")
            for half in range(2):
                nc.scalar.dma_start(
                    out=qscale_sb[half * D : (half + 1) * D, 0:1],
                    in_=qs_d[:].rearrange("(d one) -> d one", one=1),
                )
                nc.scalar.dma_start(
                    out=kscale_sb[half * D : (half + 1) * D, 0:1],
                    in_=ks_d[:].rearrange("(d one) -> d one", one=1),
                )

            # ============ phase A: per-head preprocess ============
            # The ACT table set is pinned to natural_log_exp_and_others
            # (see _pin_act_tables), so Ln/Exp never thrash table loads
            # and can be emitted per head.  q path runs on DVE + sync
            # queue; k path on GpSimd + SWDGE queue so they overlap.
            def preprocess(h):
                # head 0 is the startup critical path: spread its DMAs over
                # the otherwise-idle GpSimd/Scalar queues so the Sync queue
                # doesn't serialize everything before the first matmul
                kq_dma = nc.sync
                kt_dma = nc.sync
                xq = stagep.tile([P, NT * D], FP32, tag="stage", name=f"xq{h}")
                nc.sync.dma_start(
                    out=xq.rearrange("p (t d) -> p t d", d=D),
                    in_=q_d[h].rearrange("(t p) d -> p t d", p=P),
                )
                xk = stagep.tile([P, NT * D], FP32, tag="stage", name=f"xk{h}")
                kq_dma.dma_start(
                    out=xk.rearrange("p (t d) -> p t d", d=D),
                    in_=k_d[h].rearrange("(t p) d -> p t d", p=P),
                )
                # sum-of-squares for q (DVE) and k (square on GpSimd,
                # reduce on DVE: GpSimd cannot reduce along free axis)
                sqq = sqp.tile([P, NT * D], FP32, tag="sq", name=f"sqq{h}")
                nc.vector.tensor_mul(sqq[:], xq[:], xq[:])
                sqk = sqp.tile([P, NT * D], FP32, tag="sqk", name=f"sqk{h}")
                (nc.gpsimd if h == 0 else nc.vector).tensor_mul(sqk[:], xk[:], xk[:])
                ss = ssp.tile([P, 2 * NT], FP32, tag="ss", name=f"ss{h}")
                nc.vector.tensor_reduce(
                    out=ss[:, 0:NT],
                    in_=sqq.rearrange("p (t d) -> p t d", d=D),
                    axis=mybir.AxisListType.X,
                    op=ALU.add,
                )
                nc.vector.tensor_reduce(
                    out=ss[:, NT : 2 * NT],
                    in_=sqk.rearrange("p (t d) -> p t d", d=D),
                    axis=mybir.AxisListType.X,
                    op=ALU.add,
                )
                # rsqrt(ss) on DVE: Quake-style magic init + 2 Newton
                # iterations (keeps ScalarE exp-only -> one ACT table load
                # for the whole kernel; Rsqrt ACT table is banned anyway).
                rs = ssp.tile([P, 2 * NT], FP32, tag="ss", name=f"rs{h}")
                rsi = rs.bitcast(mybir.dt.int32)
                nc.vector.tensor_scalar(
                    rsi, ss.bitcast(mybir.dt.int32), 1, None, ALU.arith_shift_right
                )
                # i = MAGIC - (i >> 1); the +-64 fp32 rounding of the int
                # value only perturbs the initial guess, Newton absorbs it
                nc.vector.tensor_scalar(
                    rsi, rsi, -1.0, float(0x5F3759DF), ALU.mult, ALU.add
                )
                tnw = ssp.tile([P, 2 * NT], FP32, tag="ss", name=f"tnw{h}")
                for _ in range(2):
                    nc.vector.tensor_mul(tnw[:], rs[:], rs[:])
                    nc.vector.tensor_mul(tnw[:], tnw[:], ss[:])
                    nc.vector.tensor_scalar(
                        tnw[:], tnw[:], -0.5, 1.5, ALU.mult, ALU.add
                    )
                    nc.vector.tensor_mul(rs[:], rs[:], tnw[:])

                xts = {}
                for which, xs, eng, dmae, off in (
                    ("q", xq, nc.vector, nc.sync, 0),
                    ("k", xk, nc.gpsimd if h == 0 else nc.vector, kq_dma, NT),
                ):
                    xn = qnp.tile([P, NT * D], BF16, tag="qn", name=f"xn_{which}{h}")
                    rs_b = rs[:, off : off + NT].rearrange(
                        "p (t one) -> p t one", one=1
                    ).broadcast_to([P, NT, D])
                    eng.tensor_mul(
                        xn.rearrange("p (t d) -> p t d", d=D),
                        xs.rearrange("p (t d) -> p t d", d=D),
                        rs_b,
                    )
                    scratch = dramp.tile(
                        [S, P], BF16, tag="scratch", name=f"sc_{which}{h}"
                    )
                    # both 64-col halves: the duplicate feeds the K=128
                    # doubled-contraction matmul
                    for half in range(2):
                        dmae.dma_start(
                            out=scratch.rearrange("(t p) c -> p t c", p=P)[
                                :, :, half * D : (half + 1) * D
                            ],
                            in_=xn.rearrange("p (t d) -> p t d", d=D),
                        )
                    pool = qtp if which == "q" else ktp
                    xt = pool.tile([P, S], BF16, tag=f"{which}T", name=f"{which}T{h}")
                    (nc.sync if which == "q" else kt_dma).dma_start_transpose(
                        out=xt[:], in_=scratch[:]
                    )
                    nc.vector.tensor_scalar(
                        xt[:],
                        xt[:],
                        (qscale_sb if which == "q" else kscale_sb)[:, 0:1],
                        None,
                        ALU.mult,
                    )
                    xts[which] = xt

                vs = stagep.tile([P, NT * D], FP32, tag="stage", name=f"vs{h}")
                kq_dma.dma_start(
                    out=vs.rearrange("p (t d) -> p t d", d=D),
                    in_=v_d[h].rearrange("(t p) d -> p t d", p=P),
                )
                vb = vbp.tile([P, NT * (D + 1)], BF16, tag="vb", name=f"vb{h}")
                nc.vector.tensor_copy(
                    vb.rearrange("p (t c) -> p t c", c=D + 1)[:, :, 0:D],
                    vs.rearrange("p (t d) -> p t d", d=D),
                )
                nc.gpsimd.memset(
                    vb.rearrange("p (t c) -> p t c", c=D + 1)[:, :, D : D + 1], 1.0
                )
                mbias = mbp.tile([P, NT], FP32, tag="mb", name=f"mb{h}")
                kt_dma.dma_start(
                    out=mbias[:], in_=mb_d[h].rearrange("(t p) -> p t", p=P)
                )
                return xts["q"], xts["k"], vb, mbias

            def attention_head(h, qT, kT, vb, mbias, mid_hook=None):
                for ih in range(2):
                    if ih == 1 and mid_hook is not None:
                        mid_hook()
                    ilo = ih * HALF
                    njb = (ilo + HALF) // P  # 8 or 16
                    oTh = otp.tile([D + 1, HALF], FP32, tag="oT", name=f"oT{h}_{ih}")
                    ce = ilo + HALF
                    for jb in range(njb):
                        # S^T with K=128 over BOTH duplicated halves of
                        # qT/kT: computes 2x the dot product (halved via the
                        # ACT scale).  Full-K matmuls keep the PE activity
                        # monitor warm (K=64 row-group matmuls don't count
                        # as PE-busy and pin the clock at 1.2 GHz).
                        cs = max(jb * P, ilo)
                        W = ce - cs  # <= 1024: one 2-bank S^T tile per jb
                        st = stp.tile([P, W], FP32, tag="st", name=f"st{h}_{ih}_{jb}")
                        n0 = cs
                        while n0 < ce:
                            w = min(512, ce - n0)
                            nc.tensor.matmul(
                                st[:, n0 - cs : n0 - cs + w],
                                kT[:, jb * P : (jb + 1) * P],
                                qT[:, n0 : n0 + w],
                                start=True,
                                stop=True,
                            )
                            n0 += w
                        has_diag = cs == jb * P
                        vslice = vb[:, jb * (D + 1) : (jb + 1) * (D + 1)]
                        # one big exp per jb tail (fewer ACT overheads)
                        pT = ptp.tile([P, W], BF16, tag="pT", name=f"pT{h}_{ih}_{jb}")
                        nc.scalar.activation(
                            pT[:],
                            st[:],
                            AF.Exp,
                            scale=COSINE_SIM_SCALE / 2.0,
                            bias=mbias[:, jb : jb + 1],
                        )
                        if has_diag:
                            pTd = dtp.tile(
                                [P, P], BF16, tag="pTd", name=f"pTd{h}_{ih}_{jb}"
                            )
                            nc.gpsimd.tensor_mul(pTd[:], pT[:, 0:P], tri[:])
                        pv = []  # (n0, w, rhs)
                        n0 = cs
                        while n0 < ce:
                            rel = n0 - ilo
                            w = min(ilo + (rel // 512 + 1) * 512, ce) - n0
                            if has_diag and n0 == cs:
                                pv.append((n0, P, None))  # masked diag
                                if w > P:
                                    pv.append((n0 + P, w - P, pT[:, P : P + w - P]))
                            else:
                                pv.append((n0, w, pT[:, n0 - cs : n0 - cs + w]))
                            n0 += w
                        if jb != 0:
                            # diag last: keeps the gpsimd mask off the
                            # PV critical path.  jb==0 must stay in-order
                            # so every bank's first write carries start=True
                            pv.sort(key=lambda x: x[2] is None)
                        for n0, w, rhs in pv:
                            rel = n0 - ilo
                            bank = rel // 512
                            last_jb = (ilo + 512 * bank + 511) // P
                            nc.tensor.matmul(
                                oTh[:, rel : rel + w],
                                vslice,
                                rhs if rhs is not None else pTd[:],
                                start=(jb == 0),
                                stop=(jb == last_jb),
                                skip_group_check=True,
                            )

                    # ---- epilogue for this half ----
                    oT_sb = otsbp.tile(
                        [D + 1, HALF], FP32, tag="otsb", name=f"otsb{h}_{ih}"
                    )
                    # per-bank drain: bank 0's copy overlaps the tail PVs
                    # still accumulating into bank 1
                    nc.vector.tensor_copy(oT_sb[:, 0:512], oTh[:, 0:512])
                    nc.vector.tensor_copy(oT_sb[:, 512:HALF], oTh[:, 512:HALF])
                    osb = osbp.tile([P, HALF // 2], FP32, tag="osb", name=f"osb{h}_{ih}")
                    nt_h = HALF // P  # 8 tiles per half
                    for ib in range(nt_h):
                        tp = stp.tile([P, D + 1], FP32, tag="st", name=f"tp{h}_{ih}_{ib}")
                        nc.tensor.transpose(
                            tp[:],
                            oT_sb[:, ib * P : (ib + 1) * P],
                            ident[0 : D + 1, 0 : D + 1],
                        )
                        rec = recp.tile([P, 1], FP32, tag="rec", name=f"rec{h}_{ih}_{ib}")
                        nc.vector.reciprocal(rec[:], tp[:, D : D + 1])
                        nc.vector.tensor_scalar(
                            osb[:, ib * D : (ib + 1) * D],
                            tp[:, 0:D],
                            rec[:, 0:1],
                            None,
                            ALU.mult,
                        )
                    nc.sync.dma_start(
                        out=out_d[h].rearrange("(t p) d -> p t d", p=P)[
                            :, ih * nt_h : (ih + 1) * nt_h, :
                        ],
                        in_=osb.rearrange("p (t d) -> p t d", d=D),
                    )

            # pipeline: preprocess(h+1) is emitted between the two halves
            # of attention(h): its DVE ops land after half-0's epilogue in
            # the DVE FIFO (so oTh frees fast) but still a half-head early
            pre = {0: preprocess(0)}
            for h in range(HPC):

                def hook(hn=h + 1):
                    if hn < HPC:
                        pre[hn] = preprocess(hn)

                attention_head(h, *pre.pop(h), mid_hook=hook)

    nc.compile()
    return nc


_NC_CACHE = None


def kernel(q, k, v, q_scale, k_scale, mask):
    global _NC_CACHE
    q = np.asarray(q, dtype=np.float32)
    k = np.asarray(k, dtype=np.float32)
    v = np.asarray(v, dtype=np.float32)
    q_scale = np.asarray(q_scale, dtype=np.float32)
    k_scale = np.asarray(k_scale, dtype=np.float32)
    mask = np.asarray(mask)

    qf = q.reshape(B * H, S, D)
    kf = k.reshape(B * H, S, D)
    vf = v.reshape(B * H, S, D)
    # additive key-padding bias per (b,h) row, matching reference's where()
    mbias_bh = np.where(mask, 0.0, MASK_NEG).astype(np.float32)  # [B, S]

    if _NC_CACHE is None:
        _NC_CACHE = build_nc()
    nc = _NC_CACHE

    in_maps = []
    for c in range(N_CORES):
        heads = list(range(c * HPC, (c + 1) * HPC))
        in_maps.append(
            {
                "q": np.ascontiguousarray(qf[heads]),
                "k": np.ascontiguousarray(kf[heads]),
                "v": np.ascontiguousarray(vf[heads]),
                "q_scale": q_scale,
                "k_scale": k_scale,
                "mbias": np.ascontiguousarray(
                    np.stack([mbias_bh[bh // H] for bh in heads])
                ),
            }
        )

    res = run_bass_kernel_spmd(nc, in_maps, core_ids=list(range(N_CORES)))
    out = np.stack([r["out"] for r in res.results])  # [8, 4, S, D]
    return out.reshape(B, H, S, D).astype(np.float32)
